# revision 1
# baseline (speedup 1.0000x reference)
"""Causal attention with ALiBi for nn_CausalAttention (B=4, T=2048, C=1024,
16 heads) on 8 TRN2 NeuronCores.

Sharding: batch (4) x head-group (2 groups of 8 heads) -> 8 cores.
Each core computes, for its batch b and head group g:
  qT/kT = (Wg.T @ x.T) projections in [d, t] layout, v in [t, d] layout,
  per head: sT[j, i] = qk/8 + slope*(j - i) via an augmented one-hot bias
  matmul (per-column -slope*i, numerically cancels in softmax) plus an ACT
  exp bias of +slope*j (exact fp32); causal masking by gpsimd affine_select
  (also kills Inf from masked overflow); PV with an appended ones column
  gives the softmax denominator; normalization via vector.reciprocal +
  gpsimd partition_broadcast; final y_partial = oT.T @ Wo_rows.
Host sums the two head-group partials per batch.

Matmuls run in float32r (TF32-like, ~1e-3 rel); probs/V in bf16.
"""

import math

import numpy as np

import concourse.bass as bass
import concourse.mybir as mybir
import concourse.tile as tile
from concourse import bacc
from concourse.bass_utils import run_bass_kernel_spmd

B, T, C = 4, 2048, 1024
NH, HD = 16, 64
NHC = 8  # heads per core
BLOCK_SIZE = 2048
NJB = T // 128  # 16 j-blocks
NCH = T // 512  # 4 i-chunks
P = 128

f32 = mybir.dt.float32
f32r = mybir.dt.float32r
bf16 = mybir.dt.bfloat16

LAST_RESULTS = None
_NC_CACHE = None


def get_slopes(n):
    def pow2(n):
        start = 2 ** (-(2 ** (-(math.log2(n) - 3))))
        return [start * start**i for i in range(n)]

    if math.log2(n).is_integer():
        return pow2(n)
    c = 2 ** math.floor(math.log2(n))
    return pow2(c) + get_slopes(2 * c)[0::2][: n - c]


# compact pT tile index: tiles (jb, c) with c >= jb//4
_PT_OFFS = []
_o = 0
for _jb in range(NJB):
    _PT_OFFS.append(_o)
    _o += NCH - _jb // 4
NPT = _o  # 40


def build_kernel():
    nc = bacc.Bacc("TRN2", target_bir_lowering=False, debug=False, num_devices=8)

    xT_d = nc.dram_tensor("xT", [C, T], f32, kind="ExternalInput").ap()
    wq_d = nc.dram_tensor("wq", [C, 512], f32, kind="ExternalInput").ap()
    wk_d = nc.dram_tensor("wk", [C, 512], f32, kind="ExternalInput").ap()
    wv_d = nc.dram_tensor("wv", [C, 512], f32, kind="ExternalInput").ap()
    wo_d = nc.dram_tensor("wo", [512, C], f32, kind="ExternalInput").ap()
    qaug_d = nc.dram_tensor("qaugb", [8, NHC, T], bf16, kind="ExternalInput").ap()
    kaug_d = nc.dram_tensor("kaugb", [8, NHC, T], bf16, kind="ExternalInput").ap()
    biasj_d = nc.dram_tensor("biasj", [P, NHC, NJB], f32, kind="ExternalInput").ap()
    y_d = nc.dram_tensor("y", [T, C], f32, kind="ExternalOutput").ap()

    xT_r = xT_d.rearrange("(cb p) t -> p cb t", p=P)  # [128, 8, 2048]
    wq_r = wq_d.rearrange("(cb p) m -> p cb m", p=P)  # [128, 8, 512]
    wk_r = wk_d.rearrange("(cb p) m -> p cb m", p=P)
    wv_r = wv_d.rearrange("(cb p) m -> p cb m", p=P)
    wo_r = wo_d.rearrange("(mb p) n -> p mb n", p=P)  # [128, 4, 1024]
    y_r = y_d.rearrange("(tb p) c -> p tb c", p=P)  # [128, 16, 1024]

    with tile.TileContext(nc) as tc:
        with (
            tc.tile_pool(name="persist", bufs=1) as persist,
            tc.tile_pool(name="work", bufs=2) as work,
            tc.tile_pool(name="psA", bufs=2, space="PSUM") as psA,
            tc.tile_pool(name="psB", bufs=2, space="PSUM") as psB,
            tc.tile_pool(name="psC", bufs=2, space="PSUM") as psC,
        ):
            # ---- persistent tiles ----
            # qT2/kT2: per head h, rows 0-63 = head data (d), rows 64-71 =
            # augmented bias rows; K=72 matmul contracts both at once.
            qT2 = persist.tile([72, NHC, T], bf16)
            kT2 = persist.tile([72, NHC, T], bf16)
            vaug = persist.tile([P, NJB, NHC, 66], bf16)
            oT = persist.tile([P, 4, T], bf16)
            biasj = persist.tile([P, NHC, NJB], f32)

            nc.gpsimd.memset(vaug[:, :, :, 64:66], 1.0)
            nc.sync.dma_start(biasj[:], biasj_d[:])
            # aug rows: kT2 row 64+r of head h is 1.0 iff r == h;
            # qT2 row 64+r of every head = -slope_r * i
            nc.sync.dma_start(kT2[64:72, :, :], kaug_d[:])
            nc.sync.dma_start(qT2[64:72, :, :], qaug_d[:])

            # ---- fused projections + attention ----
            # v first (vaug must be complete before the first PV); then per
            # head-pair m: project q/k for pair m, then emit QK/PV for its
            # heads, software-pipelined with lag 1 so the exp/select tail of
            # each head hides under the next head's work.
            wst_cm = tc.tile_pool(name="wst", bufs=2)
            wst = wst_cm.__enter__()
            xp1_cm = tc.tile_pool(name="xp1", bufs=2)
            xp1 = xp1_cm.__enter__()
            xr1_cm = tc.tile_pool(name="xr1", bufs=9)
            xr1 = xr1_cm.__enter__()
            wqk_cm = tc.tile_pool(name="wqk", bufs=1)
            wqk = wqk_cm.__enter__()

            def load_x_chunk(tck):
                xts = []
                for c in range(8):
                    x32 = xp1.tile([P, 512], f32, tag="x32")
                    nc.sync.dma_start(x32[:], xT_r[:, c, bass.ts(tck, 512)])
                    xtr = xr1.tile([P, 512], bf16, tag="xtr")
                    nc.vector.tensor_copy(xtr[:], x32[:])
                    xts.append(xtr)
                return xts

            # --- v projection ---
            with (
                tc.tile_pool(name="wvp", bufs=1) as wvp,
                tc.tile_pool(name="wvst", bufs=2) as wvst,
            ):
                wvr = wvp.tile([P, 8, 512], bf16)
                for c in range(8):
                    w32 = wvst.tile([P, 512], f32, tag="w32")
                    nc.sync.dma_start(w32[:], wv_r[:, c, :])
                    nc.vector.tensor_copy(wvr[:, c, :], w32[:])
                for tck in range(NCH):
                    xts = load_x_chunk(tck)
                    for tb in range(4):
                        psv = psB.tile([P, 512], f32, tag="pb")
                        for c in range(8):
                            nc.tensor.matmul(
                                psv[:],
                                xts[c][:, bass.ts(tb, P)],
                                wvr[:, c, :],
                                start=(c == 0),
                                stop=(c == 7),
                            )
                        nc.vector.tensor_copy(
                            vaug[:, 4 * tck + tb, :, 0:64],
                            psv[:].rearrange("p (h d) -> p h d", h=NHC),
                        )

            def project_pair(m):
                # load this pair's weight columns (bf16 chunks)
                wqm = wqk.tile([P, 8, P], bf16, tag="wqm")
                wkm = wqk.tile([P, 8, P], bf16, tag="wkm")
                for c in range(8):
                    wq32 = wst.tile([P, P], f32, tag="wc32")
                    nc.sync.dma_start(wq32[:], wq_r[:, c, bass.ts(m, P)])
                    nc.vector.tensor_copy(wqm[:, c, :], wq32[:])
                    wk32 = wst.tile([P, P], f32, tag="wc32")
                    nc.sync.dma_start(wk32[:], wk_r[:, c, bass.ts(m, P)])
                    nc.vector.tensor_copy(wkm[:, c, :], wk32[:])
                for tck in range(NCH):
                    xts = load_x_chunk(tck)
                    psq = psB.tile([P, 512], f32, tag="pb")
                    psk = psB.tile([P, 512], f32, tag="pb")
                    for c in range(8):
                        nc.tensor.matmul(
                            psq[:],
                            wqm[:, c, :],
                            xts[c][:],
                            start=(c == 0),
                            stop=(c == 7),
                        )
                        nc.tensor.matmul(
                            psk[:],
                            wkm[:, c, :],
                            xts[c][:],
                            start=(c == 0),
                            stop=(c == 7),
                        )
                    qstag = work.tile([P, 512], bf16, tag="qkstag")
                    kstag = work.tile([P, 512], bf16, tag="qkstag")
                    nc.vector.tensor_copy(qstag[:], psq[:])
                    nc.vector.tensor_copy(kstag[:], psk[:])
                    tsl = bass.ts(tck, 512)
                    nc.sync.dma_start(qT2[0:64, 2 * m, tsl], qstag[0:64, :])
                    nc.sync.dma_start(qT2[0:64, 2 * m + 1, tsl], qstag[64:128, :])
                    nc.sync.dma_start(kT2[0:64, 2 * m, tsl], kstag[0:64, :])
                    nc.sync.dma_start(kT2[0:64, 2 * m + 1, tsl], kstag[64:128, :])

            ptp_cm = tc.tile_pool(name="ptp", bufs=2)
            ptp = ptp_cm.__enter__()
            pT_of = {}

            def emit_qk(h):
                pT = ptp.tile([P, NPT, 512], bf16, tag="pT")
                pT_of[h] = pT
                for jb in range(NJB):
                    c0 = jb // 4
                    nact = NCH - c0
                    idx0 = _PT_OFFS[jb]
                    # sub-groups of <=2 chunks so QK can run ahead of exp
                    for g0 in range(0, nact, 2):
                        ng = min(2, nact - g0)
                        ssum = psA.tile([P, 2, 512], f32, tag="ssum")
                        for ci in range(ng):
                            c = c0 + g0 + ci
                            nc.tensor.matmul(
                                ssum[:, ci, :],
                                kT2[:, h, bass.ts(jb, P)],
                                qT2[:, h, bass.ts(c, 512)],
                                start=True,
                                stop=True,
                            )
                        nc.scalar.activation(
                            pT[:, idx0 + g0 : idx0 + g0 + ng, :],
                            ssum[:, 0:ng, :],
                            mybir.ActivationFunctionType.Exp,
                            bias=biasj[:, h, jb : jb + 1],
                            scale=1.0,
                        )
                    # causal mask on the diagonal tile (c == c0):
                    # keep where i - j >= 0 ; i = 512*c0 + f, j = 128*jb + p
                    nc.gpsimd.affine_select(
                        pT[:, idx0, :],
                        pT[:, idx0, :],
                        pattern=[[1, 512]],
                        compare_op=mybir.AluOpType.is_ge,
                        fill=0.0,
                        base=512 * c0 - 128 * jb,
                        channel_multiplier=-1,
                    )

            def emit_pv(h):
                hp = (h % 2) * 64
                hm = h // 2
                pT = pT_of.pop(h)
                for c in range(NCH):
                    pot = psC.tile([65, 512], f32, tag="pot")
                    njb = 4 * c + 4
                    for jb in range(njb):
                        nc.tensor.matmul(
                            pot[:],
                            vaug[:, jb, h, 0:65],
                            pT[:, _PT_OFFS[jb] + (c - jb // 4), :],
                            start=(jb == 0),
                            stop=(jb == njb - 1),
                        )
                    # copy out fast to release the PSUM bank, then normalize
                    # off the PV critical path.
                    potsb = work.tile([65, 512], f32, tag="potsb")
                    nc.vector.tensor_copy(potsb[:], pot[:])
                    # spread the 512 rowsums across 128 partitions so the
                    # reciprocal uses all DVE lanes (26ns vs 3.3us)
                    rs128 = work.tile([P, 4], f32, tag="rs128")
                    nc.sync.dma_start(rs128[:], potsb[64:65, :])
                    nc.vector.reciprocal(rs128[:], rs128[:])
                    srecip = persist.tile([1, 512], f32, tag="srecip")
                    nc.sync.dma_start(srecip[:], rs128[:])
                    bcast = persist.tile([64, 512], f32, tag="bcast")
                    nc.gpsimd.partition_broadcast(bcast[:], srecip[:])
                    nc.vector.tensor_tensor(
                        oT[hp : hp + 64, hm, bass.ts(c, 512)],
                        potsb[0:64, :],
                        bcast[:],
                        mybir.AluOpType.mult,
                    )

            for m in range(4):
                project_pair(m)
                emit_qk(2 * m)
                if m > 0:
                    emit_pv(2 * m - 1)
                emit_qk(2 * m + 1)
                emit_pv(2 * m)
            emit_pv(NHC - 1)

            ptp_cm.__exit__(None, None, None)
            wqk_cm.__exit__(None, None, None)
            xr1_cm.__exit__(None, None, None)
            xp1_cm.__exit__(None, None, None)
            wst_cm.__exit__(None, None, None)

            # ---- output projection ----
            with (
                tc.tile_pool(name="wop", bufs=1) as wop,
                tc.tile_pool(name="wst3", bufs=2) as wst3,
                tc.tile_pool(name="ypool", bufs=2) as ypool,
            ):
                wor = wop.tile([P, 4, C], bf16)
                for m in range(4):
                    wo32 = wst3.tile([P, C], f32, tag="wo32")
                    nc.sync.dma_start(wo32[:], wo_r[:, m, :])
                    nc.vector.tensor_copy(wor[:, m, :], wo32[:])

                for tb in range(NJB):
                    for cc in range(2):
                        psy = psB.tile([P, 512], f32, tag="pb")
                        for m in range(4):
                            nc.tensor.matmul(
                                psy[:],
                                oT[:, m, bass.ts(tb, P)],
                                wor[:, m, bass.ts(cc, 512)],
                                start=(m == 0),
                                stop=(m == 3),
                            )
                        ysb = ypool.tile([P, 512], f32, tag="ysb")
                        nc.vector.tensor_copy(ysb[:], psy[:])
                        nc.sync.dma_start(y_r[:, tb, bass.ts(cc, 512)], ysb[:])

    nc.compile()
    return nc


def kernel(x, Wq, Wk, Wv, Wo):
    global LAST_RESULTS, _NC_CACHE
    x = np.asarray(x, dtype=np.float32)
    Wq = np.asarray(Wq, dtype=np.float32)
    Wk = np.asarray(Wk, dtype=np.float32)
    Wv = np.asarray(Wv, dtype=np.float32)
    Wo = np.asarray(Wo, dtype=np.float32)

    slopes = np.asarray(get_slopes(NH), dtype=np.float32)
    ii = np.arange(T, dtype=np.float64)
    pp = np.arange(P, dtype=np.float64)

    if _NC_CACHE is None:
        _NC_CACHE = build_kernel()
    nc = _NC_CACHE

    in_maps = []
    for core in range(8):
        b, g = core // 2, core % 2
        hsl = slice(g * 512, (g + 1) * 512)
        core_slopes = slopes[g * NHC : (g + 1) * NHC].astype(np.float64)
        import ml_dtypes

        qaug1 = (-core_slopes[:, None] * ii[None, :]).astype(ml_dtypes.bfloat16)
        qaugb = np.ascontiguousarray(
            np.broadcast_to(qaug1[:, None, :], (8, NHC, T))
        )
        kaugb = np.zeros((8, NHC, T), ml_dtypes.bfloat16)
        for h in range(NHC):
            kaugb[h, h, :] = ml_dtypes.bfloat16(1.0)
        biasj = np.zeros((P, NHC, NJB), np.float32)
        for h in range(NHC):
            for jb in range(NJB):
                biasj[:, h, jb] = (core_slopes[h] * (128 * jb + pp)).astype(np.float32)
        in_maps.append(
            {
                "xT": np.ascontiguousarray(x[b].T),
                "wq": np.ascontiguousarray(Wq[:, hsl]) * np.float32(0.125),
                "wk": np.ascontiguousarray(Wk[:, hsl]),
                "wv": np.ascontiguousarray(Wv[:, hsl]),
                "wo": np.ascontiguousarray(Wo[hsl, :]),
                "qaugb": qaugb,
                "kaugb": kaugb,
                "biasj": biasj,
            }
        )

    res = run_bass_kernel_spmd(nc, in_maps, list(range(8)))
    LAST_RESULTS = res
    out = np.empty((B, T, C), dtype=np.float32)
    for b in range(B):
        out[b] = res.results[2 * b]["y"] + res.results[2 * b + 1]["y"]
    return out



# revision 5
# speedup vs baseline: 1.4546x; 1.4546x over previous
"""Causal attention with ALiBi for nn_CausalAttention (B=4, T=2048, C=1024,
16 heads) on 8 TRN2 NeuronCores.

Sharding: batch (4) x head-parity (2 groups of 8 heads, interleaved so the
two groups have matching slope magnitudes) -> 8 cores.  Core (b, g) handles
heads {g, g+2, ..., g+14} of batch b.

Per core:
  single-pass projections: x chunk loaded once, q/k/v projected per chunk
  (qT/kT in [d, t] layout, v in [t, d]);
  per head-slot s: sT[j, i] = qk/8 + slope*(j - i) via augmented one-hot
  bias matmul plus ACT exp bias of +slope*j; ALiBi decay makes keys beyond
  a per-slot window numerically zero, so score tiles outside the band
  W_SLOT[s] are skipped entirely (QK, exp, and PV); causal masking by
  gpsimd affine_select on the (reduced-width) diagonal tiles; PV with an
  appended ones column gives the softmax denominator; normalization via
  vector.reciprocal + gpsimd partition_broadcast; y_partial = oT.T @ Wo_rows.
Host sums the two head-group partials per batch.
"""

import math

import numpy as np

import concourse.bass as bass
import concourse.mybir as mybir
import concourse.tile as tile
from concourse import bacc
from concourse.bass_utils import run_bass_kernel_spmd

B, T, C = 4, 2048, 1024
NH, HD = 16, 64
NHC = 8  # head-slots per core
BLOCK_SIZE = 2048
NJB = T // 128  # 16 j-blocks
NCH = T // 512  # 4 i-chunks
P = 128

f32 = mybir.dt.float32
bf16 = mybir.dt.bfloat16

LAST_RESULTS = None
_NC_CACHE = None


def get_slopes(n):
    def pow2(n):
        start = 2 ** (-(2 ** (-(math.log2(n) - 3))))
        return [start * start**i for i in range(n)]

    if math.log2(n).is_integer():
        return pow2(n)
    c = 2 ** math.floor(math.log2(n))
    return pow2(c) + get_slopes(2 * c)[0::2][: n - c]


# Per-slot ALiBi windows: slot k holds heads {2k, 2k+1} (parity split), the
# shallower slope s = 0.7071^(2k+2) needs W >= (11 + ln(1/s))/s for the
# dropped softmax mass to stay under ~1e-3.
W_SLOT = [24, 50, 105, 221, 463, 970, 2048, 2048]


def _cr(jb, W):
    """Kept i-chunk range [c0, c1] for j-block jb under window W."""
    return jb // 4, min(NCH - 1, (128 * jb + 127 + W) // 512)


_OFFS = []  # [slot][jb] -> compact pT index base (add c - c0)
_NPT = []
for _W in W_SLOT:
    _offs = []
    _o = 0
    for _jb in range(NJB):
        _c0, _c1 = _cr(_jb, _W)
        _offs.append(_o)
        _o += _c1 - _c0 + 1
    _OFFS.append(_offs)
    _NPT.append(_o)
NPTMAX = max(_NPT)  # 40


def build_kernel():
    nc = bacc.Bacc("TRN2", target_bir_lowering=False, debug=False, num_devices=8)

    xT_d = nc.dram_tensor("xT", [C, T], f32, kind="ExternalInput").ap()
    wq_d = nc.dram_tensor("wq", [C, 512], f32, kind="ExternalInput").ap()
    wk_d = nc.dram_tensor("wk", [C, 512], f32, kind="ExternalInput").ap()
    wv_d = nc.dram_tensor("wv", [C, 512], f32, kind="ExternalInput").ap()
    wo_d = nc.dram_tensor("wo", [512, C], f32, kind="ExternalInput").ap()
    qaug_d = nc.dram_tensor("qaugb", [8, NHC, T], bf16, kind="ExternalInput").ap()
    kaug_d = nc.dram_tensor("kaugb", [8, NHC, T], bf16, kind="ExternalInput").ap()
    biasj_d = nc.dram_tensor("biasj", [P, NHC, NJB], f32, kind="ExternalInput").ap()
    y_d = nc.dram_tensor("y", [T, C], f32, kind="ExternalOutput").ap()

    xT_r = xT_d.rearrange("(cb p) t -> p cb t", p=P)  # [128, 8, 2048]
    wq_r = wq_d.rearrange("(cb p) m -> p cb m", p=P)  # [128, 8, 512]
    wk_r = wk_d.rearrange("(cb p) m -> p cb m", p=P)
    wv_r = wv_d.rearrange("(cb p) m -> p cb m", p=P)
    wo_r = wo_d.rearrange("(mb p) n -> p mb n", p=P)  # [128, 4, 1024]
    y_r = y_d.rearrange("(tb p) c -> p tb c", p=P)  # [128, 16, 1024]

    with tile.TileContext(nc) as tc:
        with (
            tc.tile_pool(name="persist", bufs=1) as persist,
            tc.tile_pool(name="work", bufs=2) as work,
            tc.tile_pool(name="psA", bufs=2, space="PSUM") as psA,
            tc.tile_pool(name="psB", bufs=2, space="PSUM") as psB,
            tc.tile_pool(name="psC", bufs=2, space="PSUM") as psC,
        ):
            # ---- persistent tiles ----
            # qT2/kT2: per slot s, rows 0-63 = head data (d), rows 64-71 =
            # augmented bias rows; K=72 matmul contracts both at once.
            qT2 = persist.tile([72, NHC, T], bf16)
            kT2 = persist.tile([72, NHC, T], bf16)
            vaug = persist.tile([P, NJB, NHC, 66], bf16)
            oT = persist.tile([P, 4, T], bf16)
            biasj = persist.tile([P, NHC, NJB], f32)

            nc.gpsimd.memset(vaug[:, :, :, 64:66], 1.0)
            nc.sync.dma_start(biasj[:], biasj_d[:])
            # aug rows: kT2 row 64+r of slot s is 1.0 iff r == s;
            # qT2 row 64+r of every slot = -slope_r * i
            nc.sync.dma_start(kT2[64:72, :, :], kaug_d[:])
            nc.sync.dma_start(qT2[64:72, :, :], qaug_d[:])

            # ---- single-pass projections ----
            # All weights preloaded (bf16); x streamed chunk by chunk, each
            # chunk feeding v and all four q/k pairs before the next load.
            with (
                tc.tile_pool(name="wts", bufs=1) as wts,
                tc.tile_pool(name="wstage", bufs=2) as wstage,
                tc.tile_pool(name="xp1", bufs=2) as xp1,
                tc.tile_pool(name="xr1", bufs=9) as xr1,
            ):
                wqr = wts.tile([P, 8, 512], bf16, tag="wqr")
                wkr = wts.tile([P, 8, 512], bf16, tag="wkr")
                wvr = wts.tile([P, 8, 512], bf16, tag="wvr")
                for c in range(8):
                    for wsrc, wdst in ((wq_r, wqr), (wk_r, wkr), (wv_r, wvr)):
                        w32 = wstage.tile([P, 512], f32, tag="w32")
                        nc.sync.dma_start(w32[:], wsrc[:, c, :])
                        nc.vector.tensor_copy(wdst[:, c, :], w32[:])

                for tck in range(NCH):
                    xts = []
                    for c in range(8):
                        x32 = xp1.tile([P, 512], f32, tag="x32")
                        nc.sync.dma_start(x32[:], xT_r[:, c, bass.ts(tck, 512)])
                        xtr = xr1.tile([P, 512], bf16, tag="xtr")
                        nc.vector.tensor_copy(xtr[:], x32[:])
                        xts.append(xtr)
                    # v for this chunk
                    for tb in range(4):
                        psv = psB.tile([P, 512], f32, tag="pb")
                        for c in range(8):
                            nc.tensor.matmul(
                                psv[:],
                                xts[c][:, bass.ts(tb, P)],
                                wvr[:, c, :],
                                start=(c == 0),
                                stop=(c == 7),
                            )
                        nc.vector.tensor_copy(
                            vaug[:, 4 * tck + tb, :, 0:64],
                            psv[:].rearrange("p (h d) -> p h d", h=NHC),
                        )
                    # q/k for all pairs, this chunk
                    tsl = bass.ts(tck, 512)
                    for m in range(4):
                        psqk = psA.tile([P, 2, 512], f32, tag="ssum")
                        for c in range(8):
                            nc.tensor.matmul(
                                psqk[:, 0, :],
                                wqr[:, c, bass.ts(m, P)],
                                xts[c][:],
                                start=(c == 0),
                                stop=(c == 7),
                            )
                            nc.tensor.matmul(
                                psqk[:, 1, :],
                                wkr[:, c, bass.ts(m, P)],
                                xts[c][:],
                                start=(c == 0),
                                stop=(c == 7),
                            )
                        qkstag = work.tile([P, 2, 512], bf16, tag="qkstag")
                        nc.vector.tensor_copy(qkstag[:], psqk[:])
                        nc.sync.dma_start(qT2[0:64, 2 * m, tsl], qkstag[0:64, 0, :])
                        nc.sync.dma_start(qT2[0:64, 2 * m + 1, tsl], qkstag[64:128, 0, :])
                        nc.sync.dma_start(kT2[0:64, 2 * m, tsl], qkstag[0:64, 1, :])
                        nc.sync.dma_start(kT2[0:64, 2 * m + 1, tsl], qkstag[64:128, 1, :])

            # ---- attention (banded) ----
            ptp_cm = tc.tile_pool(name="ptp", bufs=2)
            ptp = ptp_cm.__enter__()
            pT_of = {}

            def emit_qk(s):
                W = W_SLOT[s]
                pT = ptp.tile([P, NPTMAX, 512], bf16, tag="pT")
                pT_of[s] = pT
                for jb in range(NJB):
                    c0, c1 = _cr(jb, W)
                    nact = c1 - c0 + 1
                    idx0 = _OFFS[s][jb]
                    # sub-groups of <=2 chunks so QK can run ahead of exp
                    for g0 in range(0, nact, 2):
                        ng = min(2, nact - g0)
                        ssum = psA.tile([P, 2, 512], f32, tag="ssum")
                        for ci in range(ng):
                            c = c0 + g0 + ci
                            nc.tensor.matmul(
                                ssum[:, ci, :],
                                kT2[:, s, bass.ts(jb, P)],
                                qT2[:, s, bass.ts(c, 512)],
                                start=True,
                                stop=True,
                            )
                        nc.scalar.activation(
                            pT[:, idx0 + g0 : idx0 + g0 + ng, :],
                            ssum[:, 0:ng, :],
                            mybir.ActivationFunctionType.Exp,
                            bias=biasj[:, s, jb : jb + 1],
                            scale=1.0,
                        )
                    # causal mask on the diagonal tile (c == c0): keep where
                    # i - j >= 0; i = 512*c0 + f, j = 128*jb + p.  Only the
                    # first 128*(jb%4)+128 columns can violate causality.
                    w = 128 * (jb % 4) + 128
                    nc.gpsimd.affine_select(
                        pT[:, idx0, 0:w],
                        pT[:, idx0, 0:w],
                        pattern=[[1, w]],
                        compare_op=mybir.AluOpType.is_ge,
                        fill=0.0,
                        base=512 * c0 - 128 * jb,
                        channel_multiplier=-1,
                    )

            def emit_pv(s):
                W = W_SLOT[s]
                hp = (s % 2) * 64
                hm = s // 2
                pT = pT_of.pop(s)
                for c in range(NCH):
                    pot = psC.tile([65, 512], f32, tag="pot")
                    jbmin = max(0, -((-(512 * c - W - 127)) // 128))
                    jbs = list(range(jbmin, 4 * c + 4))
                    for ji, jb in enumerate(jbs):
                        nc.tensor.matmul(
                            pot[:],
                            vaug[:, jb, s, 0:65],
                            pT[:, _OFFS[s][jb] + (c - jb // 4), :],
                            start=(ji == 0),
                            stop=(ji == len(jbs) - 1),
                        )
                    # copy out fast to release the PSUM bank, then normalize
                    # off the PV critical path.
                    potsb = work.tile([65, 512], f32, tag="potsb")
                    nc.vector.tensor_copy(potsb[:], pot[:])
                    # spread the 512 rowsums across 128 partitions so the
                    # reciprocal uses all DVE lanes (26ns vs 3.3us)
                    rs128 = work.tile([P, 4], f32, tag="rs128")
                    nc.sync.dma_start(rs128[:], potsb[64:65, :])
                    nc.vector.reciprocal(rs128[:], rs128[:])
                    srecip = persist.tile([1, 512], f32, tag="srecip")
                    nc.sync.dma_start(srecip[:], rs128[:])
                    bcast = persist.tile([64, 512], f32, tag="bcast")
                    nc.gpsimd.partition_broadcast(bcast[:], srecip[:])
                    nc.vector.tensor_tensor(
                        oT[hp : hp + 64, hm, bass.ts(c, 512)],
                        potsb[0:64, :],
                        bcast[:],
                        mybir.AluOpType.mult,
                    )

            for s in range(NHC):
                emit_qk(s)
                if s > 0:
                    emit_pv(s - 1)
            emit_pv(NHC - 1)

            ptp_cm.__exit__(None, None, None)

            # ---- output projection ----
            with (
                tc.tile_pool(name="wop", bufs=1) as wop,
                tc.tile_pool(name="wst3", bufs=2) as wst3,
                tc.tile_pool(name="ypool", bufs=2) as ypool,
            ):
                wor = wop.tile([P, 4, C], bf16)
                for m in range(4):
                    wo32 = wst3.tile([P, C], f32, tag="wo32")
                    nc.sync.dma_start(wo32[:], wo_r[:, m, :])
                    nc.vector.tensor_copy(wor[:, m, :], wo32[:])

                for tb in range(NJB):
                    for cc in range(2):
                        psy = psB.tile([P, 512], f32, tag="pb")
                        for m in range(4):
                            nc.tensor.matmul(
                                psy[:],
                                oT[:, m, bass.ts(tb, P)],
                                wor[:, m, bass.ts(cc, 512)],
                                start=(m == 0),
                                stop=(m == 3),
                            )
                        ysb = ypool.tile([P, 512], f32, tag="ysb")
                        nc.vector.tensor_copy(ysb[:], psy[:])
                        nc.sync.dma_start(y_r[:, tb, bass.ts(cc, 512)], ysb[:])

    nc.compile()
    return nc


def kernel(x, Wq, Wk, Wv, Wo):
    global LAST_RESULTS, _NC_CACHE
    x = np.asarray(x, dtype=np.float32)
    Wq = np.asarray(Wq, dtype=np.float32)
    Wk = np.asarray(Wk, dtype=np.float32)
    Wv = np.asarray(Wv, dtype=np.float32)
    Wo = np.asarray(Wo, dtype=np.float32)

    slopes = np.asarray(get_slopes(NH), dtype=np.float64)
    ii = np.arange(T, dtype=np.float64)
    pp = np.arange(P, dtype=np.float64)

    if _NC_CACHE is None:
        _NC_CACHE = build_kernel()
    nc = _NC_CACHE

    import ml_dtypes

    in_maps = []
    for core in range(8):
        b, g = core // 2, core % 2
        heads = [g + 2 * k for k in range(NHC)]
        cols = np.concatenate([np.arange(64 * h, 64 * (h + 1)) for h in heads])
        core_slopes = slopes[heads]

        qaug1 = (-core_slopes[:, None] * ii[None, :]).astype(ml_dtypes.bfloat16)
        qaugb = np.ascontiguousarray(
            np.broadcast_to(qaug1[:, None, :], (8, NHC, T))
        )
        kaugb = np.zeros((8, NHC, T), ml_dtypes.bfloat16)
        for h in range(NHC):
            kaugb[h, h, :] = ml_dtypes.bfloat16(1.0)
        biasj = np.zeros((P, NHC, NJB), np.float32)
        for h in range(NHC):
            for jb in range(NJB):
                biasj[:, h, jb] = (core_slopes[h] * (128 * jb + pp)).astype(np.float32)
        in_maps.append(
            {
                "xT": np.ascontiguousarray(x[b].T),
                "wq": np.ascontiguousarray(Wq[:, cols]) * np.float32(0.125),
                "wk": np.ascontiguousarray(Wk[:, cols]),
                "wv": np.ascontiguousarray(Wv[:, cols]),
                "wo": np.ascontiguousarray(Wo[cols, :]),
                "qaugb": qaugb,
                "kaugb": kaugb,
                "biasj": biasj,
            }
        )

    res = run_bass_kernel_spmd(nc, in_maps, list(range(8)))
    LAST_RESULTS = res
    out = np.empty((B, T, C), dtype=np.float32)
    for b in range(B):
        out[b] = res.results[2 * b]["y"] + res.results[2 * b + 1]["y"]
    return out


# revision 12
# speedup vs baseline: 1.4717x; 1.0117x over previous
"""Causal attention with ALiBi for nn_CausalAttention (B=4, T=2048, C=1024,
16 heads) on 8 TRN2 NeuronCores.

Sharding: batch (4) x head-parity (2 groups of 8 heads, interleaved so the
two groups have matching slope magnitudes) -> 8 cores.  Core (b, g) handles
heads {g, g+2, ..., g+14} of batch b.

Per core:
  single-pass projections: x chunk loaded once, q/k/v projected per chunk
  (qT/kT in [d, t] layout, v in [t, d]);
  per head-slot s: sT[j, i] = qk/8 + slope*(j - i) via augmented one-hot
  bias matmul plus ACT exp bias of +slope*j; ALiBi decay makes keys beyond
  a per-slot window numerically zero, so score tiles outside the band
  W_SLOT[s] are skipped entirely (QK, exp, and PV); causal masking by
  gpsimd affine_select on the (reduced-width) diagonal tiles; PV with an
  appended ones column gives the softmax denominator; normalization via
  vector.reciprocal + gpsimd partition_broadcast; y_partial = oT.T @ Wo_rows.
Host sums the two head-group partials per batch.
"""

import math

import numpy as np

import concourse.bass as bass
import concourse.mybir as mybir
import concourse.tile as tile
from concourse import bacc
from concourse.bass_utils import run_bass_kernel_spmd

B, T, C = 4, 2048, 1024
NH, HD = 16, 64
NHC = 8  # head-slots per core
BLOCK_SIZE = 2048
NJB = T // 128  # 16 j-blocks
NCH = T // 512  # 4 i-chunks
P = 128

f32 = mybir.dt.float32
bf16 = mybir.dt.bfloat16

LAST_RESULTS = None
_NC_CACHE = None


def get_slopes(n):
    def pow2(n):
        start = 2 ** (-(2 ** (-(math.log2(n) - 3))))
        return [start * start**i for i in range(n)]

    if math.log2(n).is_integer():
        return pow2(n)
    c = 2 ** math.floor(math.log2(n))
    return pow2(c) + get_slopes(2 * c)[0::2][: n - c]


# Per-slot ALiBi windows: slot k holds heads {2k, 2k+1} (parity split), the
# shallower slope s = 0.7071^(2k+2) needs W >= (11 + ln(1/s))/s for the
# dropped softmax mass to stay under ~1e-3.
W_SLOT = [24, 50, 105, 221, 463, 970, 2048, 2048]
# Slots whose window fits in one sub-diagonal 128-block use diagonal-strip
# tiles [128j x 128i] (j in {ic-1, ic} blocks) instead of [128 x 512] tiles:
# 4x less exp area.  Their qaug rows hold -slope*(128 + i mod 128) (the
# per-i offset cancels in softmax), so the exp bias is biasj[:, s, b].
STRIP_SLOTS = 3  # slots 0..2 (W <= 105 < 128)


def _cr(jb, W):
    """Kept i-chunk range [c0, c1] for j-block jb under window W."""
    return jb // 4, min(NCH - 1, (128 * jb + 127 + W) // 512)


_OFFS = []  # [slot][jb] -> compact pT index base (add c - c0)
_NPT = []
for _W in W_SLOT:
    _offs = []
    _o = 0
    for _jb in range(NJB):
        _c0, _c1 = _cr(_jb, _W)
        _offs.append(_o)
        _o += _c1 - _c0 + 1
    _OFFS.append(_offs)
    _NPT.append(_o)
NPTMAX = max(_NPT)  # 40


def build_kernel():
    nc = bacc.Bacc("TRN2", target_bir_lowering=False, debug=False, num_devices=8)

    xT_d = nc.dram_tensor("xT", [C, T], f32, kind="ExternalInput").ap()
    wq_d = nc.dram_tensor("wq", [C, 512], f32, kind="ExternalInput").ap()
    wk_d = nc.dram_tensor("wk", [C, 512], f32, kind="ExternalInput").ap()
    wv_d = nc.dram_tensor("wv", [C, 512], f32, kind="ExternalInput").ap()
    wo_d = nc.dram_tensor("wo", [512, C], f32, kind="ExternalInput").ap()
    qaug_d = nc.dram_tensor("qaugb", [8, NHC, T], bf16, kind="ExternalInput").ap()
    kaug_d = nc.dram_tensor("kaugb", [8, NHC, T], bf16, kind="ExternalInput").ap()
    biasj_d = nc.dram_tensor("biasj", [P, NHC, NJB], f32, kind="ExternalInput").ap()
    y_d = nc.dram_tensor("y", [T, C], f32, kind="ExternalOutput").ap()

    xT_r = xT_d.rearrange("(cb p) t -> p cb t", p=P)  # [128, 8, 2048]
    wq_r = wq_d.rearrange("(cb p) m -> p cb m", p=P)  # [128, 8, 512]
    wk_r = wk_d.rearrange("(cb p) m -> p cb m", p=P)
    wv_r = wv_d.rearrange("(cb p) m -> p cb m", p=P)
    wo_r = wo_d.rearrange("(mb p) n -> p mb n", p=P)  # [128, 4, 1024]
    y_r = y_d.rearrange("(tb p) c -> p tb c", p=P)  # [128, 16, 1024]

    with tile.TileContext(nc) as tc:
        with (
            tc.tile_pool(name="persist", bufs=1) as persist,
            tc.tile_pool(name="work", bufs=2) as work,
            tc.tile_pool(name="psA", bufs=2, space="PSUM") as psA,
            tc.tile_pool(name="psB", bufs=2, space="PSUM") as psB,
            tc.tile_pool(name="psC", bufs=2, space="PSUM") as psC,
        ):
            # ---- persistent tiles ----
            # qT2/kT2: per slot s, rows 0-63 = head data (d), rows 64-71 =
            # augmented bias rows; K=72 matmul contracts both at once.
            qT2 = persist.tile([72, NHC, T], bf16)
            kT2 = persist.tile([72, NHC, T], bf16)
            vaug = persist.tile([P, NJB, NHC, 66], bf16)
            oT = persist.tile([P, 4, T], bf16)
            biasj = persist.tile([P, NHC, NJB], f32)

            nc.gpsimd.memset(vaug[:, :, :, 64:66], 1.0)
            nc.sync.dma_start(biasj[:], biasj_d[:])
            # aug rows: kT2 row 64+r of slot s is 1.0 iff r == s;
            # qT2 row 64+r of every slot = -slope_r * i
            nc.sync.dma_start(kT2[64:72, :, :], kaug_d[:])
            nc.sync.dma_start(qT2[64:72, :, :], qaug_d[:])

            # ---- single-pass projections ----
            # All weights preloaded (bf16); x streamed chunk by chunk, each
            # chunk feeding v and all four q/k pairs before the next load.
            with (
                tc.tile_pool(name="wts", bufs=1) as wts,
                tc.tile_pool(name="wstage", bufs=2) as wstage,
                tc.tile_pool(name="xp1", bufs=2) as xp1,
                tc.tile_pool(name="xr1", bufs=9) as xr1,
            ):
                wqr = wts.tile([P, 8, 512], bf16, tag="wqr")
                wkr = wts.tile([P, 8, 512], bf16, tag="wkr")
                wvr = wts.tile([P, 8, 512], bf16, tag="wvr")
                # wv first: the first v matmuls only need wvr + x chunk 0
                for wsrc, wdst in ((wv_r, wvr), (wq_r, wqr), (wk_r, wkr)):
                    for c in range(8):
                        w32 = wstage.tile([P, 512], f32, tag="w32")
                        nc.sync.dma_start(w32[:], wsrc[:, c, :])
                        nc.vector.tensor_copy(wdst[:, c, :], w32[:])

                for tck in range(NCH):
                    xts = []
                    for c in range(8):
                        x32 = xp1.tile([P, 512], f32, tag="x32")
                        nc.sync.dma_start(x32[:], xT_r[:, c, bass.ts(tck, 512)])
                        xtr = xr1.tile([P, 512], bf16, tag="xtr")
                        nc.vector.tensor_copy(xtr[:], x32[:])
                        xts.append(xtr)
                    # v for this chunk
                    for tb in range(4):
                        psv = psB.tile([P, 512], f32, tag="pb")
                        for c in range(8):
                            nc.tensor.matmul(
                                psv[:],
                                xts[c][:, bass.ts(tb, P)],
                                wvr[:, c, :],
                                start=(c == 0),
                                stop=(c == 7),
                            )
                        nc.vector.tensor_copy(
                            vaug[:, 4 * tck + tb, :, 0:64],
                            psv[:].rearrange("p (h d) -> p h d", h=NHC),
                        )
                    # q/k for all pairs, this chunk
                    tsl = bass.ts(tck, 512)
                    for m in range(4):
                        psqk = psA.tile([P, 2, 512], f32, tag="ssum")
                        for c in range(8):
                            nc.tensor.matmul(
                                psqk[:, 0, :],
                                wqr[:, c, bass.ts(m, P)],
                                xts[c][:],
                                start=(c == 0),
                                stop=(c == 7),
                            )
                            nc.tensor.matmul(
                                psqk[:, 1, :],
                                wkr[:, c, bass.ts(m, P)],
                                xts[c][:],
                                start=(c == 0),
                                stop=(c == 7),
                            )
                        qkstag = work.tile([P, 2, 512], bf16, tag="qkstag")
                        nc.vector.tensor_copy(qkstag[:], psqk[:])
                        nc.sync.dma_start(qT2[0:64, 2 * m, tsl], qkstag[0:64, 0, :])
                        nc.sync.dma_start(qT2[0:64, 2 * m + 1, tsl], qkstag[64:128, 0, :])
                        nc.sync.dma_start(kT2[0:64, 2 * m, tsl], qkstag[0:64, 1, :])
                        nc.sync.dma_start(kT2[0:64, 2 * m + 1, tsl], qkstag[64:128, 1, :])

            # ---- attention (banded) ----
            ptp_cm = tc.tile_pool(name="ptp", bufs=2)
            ptp = ptp_cm.__enter__()
            pT_of = {}

            def emit_qk_strip(s):
                # 16 i-chunks of 128; chunk ic covers j-blocks {ic-1, ic}.
                pTs = work.tile([P, 16, 2, P], bf16, tag="pTs")
                pT_of[s] = pTs
                for g in range(4):
                    psE = psA.tile([P, 2, 512], f32, tag="ssum")
                    for u in range(4):
                        ic = 4 * g + u
                        for b in range(2):
                            jb = ic - 1 + b
                            if jb < 0:
                                continue
                            nc.tensor.matmul(
                                psE[:, b, u * P : (u + 1) * P],
                                kT2[:, s, bass.ts(jb, P)],
                                qT2[:, s, ic * P : (ic + 1) * P],
                                start=True,
                                stop=True,
                            )
                    for b in range(2):
                        if g == 0 and b == 0:
                            # ic=0 has no b=0 block; skip its psum region
                            nc.scalar.activation(
                                pTs[:, 1:4, 0, :],
                                psE[:, 0, P:512],
                                mybir.ActivationFunctionType.Exp,
                                bias=biasj[:, s, 0:1],
                                scale=1.0,
                            )
                        else:
                            nc.scalar.activation(
                                pTs[:, 4 * g : 4 * g + 4, b, :],
                                psE[:, b, :],
                                mybir.ActivationFunctionType.Exp,
                                bias=biasj[:, s, b : b + 1],
                                scale=1.0,
                            )
                # causal mask on all diagonal blocks at once: keep f - p >= 0
                nc.gpsimd.affine_select(
                    pTs[:, :, 1, :],
                    pTs[:, :, 1, :],
                    pattern=[[0, 16], [1, P]],
                    compare_op=mybir.AluOpType.is_ge,
                    fill=0.0,
                    base=0,
                    channel_multiplier=-1,
                )

            def emit_pv_strip(s):
                hp = (s % 2) * 64
                hm = s // 2
                pTs = pT_of.pop(s)
                for g in range(4):
                    pot = psC.tile([65, 512], f32, tag="pot")
                    for u in range(4):
                        ic = 4 * g + u
                        bs = [b for b in range(2) if ic - 1 + b >= 0]
                        for bi, b in enumerate(bs):
                            nc.tensor.matmul(
                                pot[:, u * P : (u + 1) * P],
                                vaug[:, ic - 1 + b, s, 0:65],
                                pTs[:, ic, b, :],
                                start=(bi == 0),
                                stop=(bi == len(bs) - 1),
                            )
                    potsb = work.tile([65, 512], f32, tag="potsb")
                    nc.vector.tensor_copy(potsb[:], pot[:])
                    rs128 = work.tile([P, 4], f32, tag="rs128")
                    nc.sync.dma_start(rs128[:], potsb[64:65, :])
                    nc.vector.reciprocal(rs128[:], rs128[:])
                    srecip = persist.tile([1, 512], f32, tag="srecip")
                    nc.sync.dma_start(srecip[:], rs128[:])
                    bcast = persist.tile([64, 512], f32, tag="bcast")
                    nc.gpsimd.partition_broadcast(bcast[:], srecip[:])
                    nc.vector.tensor_tensor(
                        oT[hp : hp + 64, hm, bass.ts(g, 512)],
                        potsb[0:64, :],
                        bcast[:],
                        mybir.AluOpType.mult,
                    )

            def emit_qk(s):
                W = W_SLOT[s]
                pT = ptp.tile([P, NPTMAX, 512], bf16, tag="pT")
                pT_of[s] = pT
                for jb in range(NJB):
                    c0, c1 = _cr(jb, W)
                    nact = c1 - c0 + 1
                    idx0 = _OFFS[s][jb]
                    # sub-groups of <=2 chunks so QK can run ahead of exp
                    for g0 in range(0, nact, 2):
                        ng = min(2, nact - g0)
                        ssum = psA.tile([P, 2, 512], f32, tag="ssum")
                        for ci in range(ng):
                            c = c0 + g0 + ci
                            nc.tensor.matmul(
                                ssum[:, ci, :],
                                kT2[:, s, bass.ts(jb, P)],
                                qT2[:, s, bass.ts(c, 512)],
                                start=True,
                                stop=True,
                            )
                        nc.scalar.activation(
                            pT[:, idx0 + g0 : idx0 + g0 + ng, :],
                            ssum[:, 0:ng, :],
                            mybir.ActivationFunctionType.Exp,
                            bias=biasj[:, s, jb : jb + 1],
                            scale=1.0,
                        )
                    # causal mask on the diagonal tile (c == c0): keep where
                    # i - j >= 0; i = 512*c0 + f, j = 128*jb + p.  Only the
                    # first 128*(jb%4)+128 columns can violate causality.
                    w = 128 * (jb % 4) + 128
                    nc.gpsimd.affine_select(
                        pT[:, idx0, 0:w],
                        pT[:, idx0, 0:w],
                        pattern=[[1, w]],
                        compare_op=mybir.AluOpType.is_ge,
                        fill=0.0,
                        base=512 * c0 - 128 * jb,
                        channel_multiplier=-1,
                    )

            def emit_pv(s):
                W = W_SLOT[s]
                hp = (s % 2) * 64
                hm = s // 2
                pT = pT_of.pop(s)
                for c in range(NCH):
                    pot = psC.tile([65, 512], f32, tag="pot")
                    jbmin = max(0, -((-(512 * c - W - 127)) // 128))
                    jbs = list(range(jbmin, 4 * c + 4))
                    for ji, jb in enumerate(jbs):
                        nc.tensor.matmul(
                            pot[:],
                            vaug[:, jb, s, 0:65],
                            pT[:, _OFFS[s][jb] + (c - jb // 4), :],
                            start=(ji == 0),
                            stop=(ji == len(jbs) - 1),
                        )
                    # copy out fast to release the PSUM bank, then normalize
                    # off the PV critical path.
                    potsb = work.tile([65, 512], f32, tag="potsb")
                    nc.vector.tensor_copy(potsb[:], pot[:])
                    # spread the 512 rowsums across 128 partitions so the
                    # reciprocal uses all DVE lanes (26ns vs 3.3us)
                    rs128 = work.tile([P, 4], f32, tag="rs128")
                    nc.sync.dma_start(rs128[:], potsb[64:65, :])
                    nc.vector.reciprocal(rs128[:], rs128[:])
                    srecip = persist.tile([1, 512], f32, tag="srecip")
                    nc.sync.dma_start(srecip[:], rs128[:])
                    bcast = persist.tile([64, 512], f32, tag="bcast")
                    nc.gpsimd.partition_broadcast(bcast[:], srecip[:])
                    nc.vector.tensor_tensor(
                        oT[hp : hp + 64, hm, bass.ts(c, 512)],
                        potsb[0:64, :],
                        bcast[:],
                        mybir.AluOpType.mult,
                    )

            def do_qk(s):
                (emit_qk_strip if s < STRIP_SLOTS else emit_qk)(s)

            def do_pv(s):
                (emit_pv_strip if s < STRIP_SLOTS else emit_pv)(s)

            for s in range(NHC):
                do_qk(s)
                if s > 0:
                    do_pv(s - 1)
            do_pv(NHC - 1)

            ptp_cm.__exit__(None, None, None)

            # ---- output projection ----
            with (
                tc.tile_pool(name="wop", bufs=1) as wop,
                tc.tile_pool(name="wst3", bufs=2) as wst3,
                tc.tile_pool(name="ypool", bufs=2) as ypool,
            ):
                wor = wop.tile([P, 4, C], bf16)
                for m in range(4):
                    wo32 = wst3.tile([P, C], f32, tag="wo32")
                    nc.sync.dma_start(wo32[:], wo_r[:, m, :])
                    nc.vector.tensor_copy(wor[:, m, :], wo32[:])

                for tb in range(NJB):
                    # both 512-col halves in one 2-bank psA tile so four
                    # chains stay in flight across the 2 pool bufs
                    psy = psA.tile([P, 2, 512], f32, tag="ssum")
                    for cc in range(2):
                        for m in range(4):
                            nc.tensor.matmul(
                                psy[:, cc, :],
                                oT[:, m, bass.ts(tb, P)],
                                wor[:, m, bass.ts(cc, 512)],
                                start=(m == 0),
                                stop=(m == 3),
                            )
                    ysb = ypool.tile([P, 2, 512], f32, tag="ysb")
                    nc.vector.tensor_copy(ysb[:], psy[:])
                    nc.sync.dma_start(
                        y_r[:, tb, :].rearrange("p (a b) -> p a b", a=2), ysb[:]
                    )

    nc.compile()
    return nc


def kernel(x, Wq, Wk, Wv, Wo):
    global LAST_RESULTS, _NC_CACHE
    x = np.asarray(x, dtype=np.float32)
    Wq = np.asarray(Wq, dtype=np.float32)
    Wk = np.asarray(Wk, dtype=np.float32)
    Wv = np.asarray(Wv, dtype=np.float32)
    Wo = np.asarray(Wo, dtype=np.float32)

    slopes = np.asarray(get_slopes(NH), dtype=np.float64)
    ii = np.arange(T, dtype=np.float64)
    pp = np.arange(P, dtype=np.float64)

    if _NC_CACHE is None:
        _NC_CACHE = build_kernel()
    nc = _NC_CACHE

    import ml_dtypes

    in_maps = []
    for core in range(8):
        b, g = core // 2, core % 2
        heads = [g + 2 * k for k in range(NHC)]
        cols = np.concatenate([np.arange(64 * h, 64 * (h + 1)) for h in heads])
        core_slopes = slopes[heads]

        qaug1 = (-core_slopes[:, None] * ii[None, :]).astype(np.float64)
        # strip slots: per-i offset is -slope*(128 + i mod 128) instead
        qaug1[:STRIP_SLOTS] = -core_slopes[:STRIP_SLOTS, None] * (
            128.0 + (ii[None, :] % 128.0)
        )
        qaug1 = qaug1.astype(ml_dtypes.bfloat16)
        qaugb = np.ascontiguousarray(
            np.broadcast_to(qaug1[:, None, :], (8, NHC, T))
        )
        kaugb = np.zeros((8, NHC, T), ml_dtypes.bfloat16)
        for h in range(NHC):
            kaugb[h, h, :] = ml_dtypes.bfloat16(1.0)
        biasj = np.zeros((P, NHC, NJB), np.float32)
        for h in range(NHC):
            for jb in range(NJB):
                biasj[:, h, jb] = (core_slopes[h] * (128 * jb + pp)).astype(np.float32)
        in_maps.append(
            {
                "xT": np.ascontiguousarray(x[b].T),
                "wq": np.ascontiguousarray(Wq[:, cols]) * np.float32(0.125),
                "wk": np.ascontiguousarray(Wk[:, cols]),
                "wv": np.ascontiguousarray(Wv[:, cols]),
                "wo": np.ascontiguousarray(Wo[cols, :]),
                "qaugb": qaugb,
                "kaugb": kaugb,
                "biasj": biasj,
            }
        )

    res = run_bass_kernel_spmd(nc, in_maps, list(range(8)))
    LAST_RESULTS = res
    out = np.empty((B, T, C), dtype=np.float32)
    for b in range(B):
        out[b] = res.results[2 * b]["y"] + res.results[2 * b + 1]["y"]
    return out


# revision 14
# speedup vs baseline: 1.5185x; 1.0318x over previous
"""Causal attention with ALiBi for nn_CausalAttention (B=4, T=2048, C=1024,
16 heads) on 8 TRN2 NeuronCores.

Sharding: batch (4) x head-parity (2 groups of 8 heads, interleaved so the
two groups have matching slope magnitudes) -> 8 cores.  Core (b, g) handles
heads {g, g+2, ..., g+14} of batch b.

Per core:
  single-pass projections: x chunk loaded once, q/k/v projected per chunk
  (qT/kT in [d, t] layout, v in [t, d]);
  per head-slot s: sT[j, i] = qk/8 + slope*(j - i) via augmented one-hot
  bias matmul plus ACT exp bias of +slope*j; ALiBi decay makes keys beyond
  a per-slot window numerically zero, so score tiles outside the band
  W_SLOT[s] are skipped entirely (QK, exp, and PV); causal masking by
  gpsimd affine_select on the (reduced-width) diagonal tiles; PV with an
  appended ones column gives the softmax denominator; normalization via
  vector.reciprocal + gpsimd partition_broadcast; y_partial = oT.T @ Wo_rows.
Host sums the two head-group partials per batch.
"""

import math

import numpy as np

import concourse.bass as bass
import concourse.mybir as mybir
import concourse.tile as tile
from concourse import bacc
from concourse.bass_utils import run_bass_kernel_spmd

B, T, C = 4, 2048, 1024
NH, HD = 16, 64
NHC = 8  # head-slots per core
BLOCK_SIZE = 2048
NJB = T // 128  # 16 j-blocks
NCH = T // 512  # 4 i-chunks
P = 128

f32 = mybir.dt.float32
bf16 = mybir.dt.bfloat16

LAST_RESULTS = None
_NC_CACHE = None


def get_slopes(n):
    def pow2(n):
        start = 2 ** (-(2 ** (-(math.log2(n) - 3))))
        return [start * start**i for i in range(n)]

    if math.log2(n).is_integer():
        return pow2(n)
    c = 2 ** math.floor(math.log2(n))
    return pow2(c) + get_slopes(2 * c)[0::2][: n - c]


# Per-slot ALiBi windows: slot k holds heads {2k, 2k+1} (parity split), the
# shallower slope s = 0.7071^(2k+2) needs W >= (11 + ln(1/s))/s for the
# dropped softmax mass to stay under ~1e-3.
W_SLOT = [24, 50, 105, 221, 463, 970, 2048, 2048]
# Slots whose window fits in one sub-diagonal 128-block use diagonal-strip
# tiles [128j x 128i] (j in {ic-1, ic} blocks) instead of [128 x 512] tiles:
# 4x less exp area.  Their qaug rows hold -slope*(128 + i mod 128) (the
# per-i offset cancels in softmax), so the exp bias is biasj[:, s, b].
STRIP_SLOTS = 3  # slots 0..2 (W <= 105 < 128)


def _cr(jb, W):
    """Kept i-chunk range [c0, c1] for j-block jb under window W."""
    return jb // 4, min(NCH - 1, (128 * jb + 127 + W) // 512)


_OFFS = []  # [slot][jb] -> compact pT index base (add c - c0)
_NPT = []
for _W in W_SLOT:
    _offs = []
    _o = 0
    for _jb in range(NJB):
        _c0, _c1 = _cr(_jb, _W)
        _offs.append(_o)
        _o += _c1 - _c0 + 1
    _OFFS.append(_offs)
    _NPT.append(_o)
NPTMAX = max(_NPT)  # 40


def build_kernel():
    nc = bacc.Bacc("TRN2", target_bir_lowering=False, debug=False, num_devices=8)

    xT_d = nc.dram_tensor("xT", [C, T], f32, kind="ExternalInput").ap()
    wq_d = nc.dram_tensor("wq", [C, 512], f32, kind="ExternalInput").ap()
    wk_d = nc.dram_tensor("wk", [C, 512], f32, kind="ExternalInput").ap()
    wv_d = nc.dram_tensor("wv", [C, 512], f32, kind="ExternalInput").ap()
    wo_d = nc.dram_tensor("wo", [512, C], f32, kind="ExternalInput").ap()
    qaug_d = nc.dram_tensor("qaugb", [8, NHC, T], bf16, kind="ExternalInput").ap()
    kaug_d = nc.dram_tensor("kaugb", [8, NHC, T], bf16, kind="ExternalInput").ap()
    biasj_d = nc.dram_tensor("biasj", [P, NHC, NJB], f32, kind="ExternalInput").ap()
    y_d = nc.dram_tensor("y", [T, C], f32, kind="ExternalOutput").ap()

    xT_r = xT_d.rearrange("(cb p) t -> p cb t", p=P)  # [128, 8, 2048]
    wq_r = wq_d.rearrange("(cb p) m -> p cb m", p=P)  # [128, 8, 512]
    wk_r = wk_d.rearrange("(cb p) m -> p cb m", p=P)
    wv_r = wv_d.rearrange("(cb p) m -> p cb m", p=P)
    wo_r = wo_d.rearrange("(mb p) n -> p mb n", p=P)  # [128, 4, 1024]
    y_r = y_d.rearrange("(tb p) c -> p tb c", p=P)  # [128, 16, 1024]

    with tile.TileContext(nc) as tc:
        with (
            tc.tile_pool(name="persist", bufs=1) as persist,
            tc.tile_pool(name="work", bufs=2) as work,
            tc.tile_pool(name="psA", bufs=2, space="PSUM") as psA,
            tc.tile_pool(name="psB", bufs=2, space="PSUM") as psB,
            tc.tile_pool(name="psC", bufs=2, space="PSUM") as psC,
        ):
            # ---- persistent tiles ----
            # qT2/kT2: per slot s, rows 0-63 = head data (d), rows 64-71 =
            # augmented bias rows; K=72 matmul contracts both at once.
            qT2 = persist.tile([72, NHC, T], bf16)
            kT2 = persist.tile([72, NHC, T], bf16)
            vaug = persist.tile([P, NJB, NHC, 66], bf16)
            oT = persist.tile([P, 4, T], bf16)
            biasj = persist.tile([P, NHC, NJB], f32)

            nc.gpsimd.memset(vaug[:, :, :, 64:66], 1.0)
            nc.sync.dma_start(biasj[:], biasj_d[:])
            # aug rows: kT2 row 64+r of slot s is 1.0 iff r == s;
            # qT2 row 64+r of every slot = -slope_r * i
            nc.sync.dma_start(kT2[64:72, :, :], kaug_d[:])
            nc.sync.dma_start(qT2[64:72, :, :], qaug_d[:])

            # ---- single-pass projections ----
            # All weights preloaded (bf16); x streamed chunk by chunk, each
            # chunk feeding v and all four q/k pairs before the next load.
            with (
                tc.tile_pool(name="wts", bufs=1) as wts,
                tc.tile_pool(name="wstage", bufs=2) as wstage,
                tc.tile_pool(name="xp1", bufs=2) as xp1,
                tc.tile_pool(name="xr1", bufs=9) as xr1,
            ):
                wqr = wts.tile([P, 8, 512], bf16, tag="wqr")
                wkr = wts.tile([P, 8, 512], bf16, tag="wkr")
                wvr = wts.tile([P, 8, 512], bf16, tag="wvr")
                # wv first: the first v matmuls only need wvr + x chunk 0
                for wsrc, wdst in ((wv_r, wvr), (wq_r, wqr), (wk_r, wkr)):
                    for c in range(8):
                        w32 = wstage.tile([P, 512], f32, tag="w32")
                        nc.scalar.dma_start(w32[:], wsrc[:, c, :])
                        nc.vector.tensor_copy(wdst[:, c, :], w32[:])

                for tck in range(NCH):
                    xts = []
                    for c in range(8):
                        x32 = xp1.tile([P, 512], f32, tag="x32")
                        nc.sync.dma_start(x32[:], xT_r[:, c, bass.ts(tck, 512)])
                        xtr = xr1.tile([P, 512], bf16, tag="xtr")
                        nc.vector.tensor_copy(xtr[:], x32[:])
                        xts.append(xtr)
                    # interleave one v chain (psB) with one q/k chain (psA)
                    # per step so the PE never drains a pool dry (keeps the
                    # HAM clock warm: isolated 8-MM chains ran at 1.2 GHz)
                    tsl = bass.ts(tck, 512)
                    for i in range(4):
                        psv = psB.tile([P, 512], f32, tag="pb")
                        for c in range(8):
                            nc.tensor.matmul(
                                psv[:],
                                xts[c][:, bass.ts(i, P)],
                                wvr[:, c, :],
                                start=(c == 0),
                                stop=(c == 7),
                            )
                        nc.vector.tensor_copy(
                            vaug[:, 4 * tck + i, :, 0:64],
                            psv[:].rearrange("p (h d) -> p h d", h=NHC),
                        )
                        psqk = psA.tile([P, 2, 512], f32, tag="ssum")
                        for c in range(8):
                            nc.tensor.matmul(
                                psqk[:, 0, :],
                                wqr[:, c, bass.ts(i, P)],
                                xts[c][:],
                                start=(c == 0),
                                stop=(c == 7),
                            )
                            nc.tensor.matmul(
                                psqk[:, 1, :],
                                wkr[:, c, bass.ts(i, P)],
                                xts[c][:],
                                start=(c == 0),
                                stop=(c == 7),
                            )
                        qkstag = work.tile([P, 2, 512], bf16, tag="qkstag")
                        nc.vector.tensor_copy(qkstag[:], psqk[:])
                        nc.scalar.dma_start(qT2[0:64, 2 * i, tsl], qkstag[0:64, 0, :])
                        nc.scalar.dma_start(qT2[0:64, 2 * i + 1, tsl], qkstag[64:128, 0, :])
                        nc.scalar.dma_start(kT2[0:64, 2 * i, tsl], qkstag[0:64, 1, :])
                        nc.scalar.dma_start(kT2[0:64, 2 * i + 1, tsl], qkstag[64:128, 1, :])

            # ---- attention (banded) ----
            ptp_cm = tc.tile_pool(name="ptp", bufs=2)
            ptp = ptp_cm.__enter__()
            pT_of = {}

            def emit_qk_strip(s):
                # 16 i-chunks of 128; chunk ic covers j-blocks {ic-1, ic}.
                pTs = work.tile([P, 16, 2, P], bf16, tag="pTs")
                pT_of[s] = pTs
                for g in range(4):
                    psE = psA.tile([P, 2, 512], f32, tag="ssum")
                    for u in range(4):
                        ic = 4 * g + u
                        for b in range(2):
                            jb = ic - 1 + b
                            if jb < 0:
                                continue
                            nc.tensor.matmul(
                                psE[:, b, u * P : (u + 1) * P],
                                kT2[:, s, bass.ts(jb, P)],
                                qT2[:, s, ic * P : (ic + 1) * P],
                                start=True,
                                stop=True,
                            )
                    for b in range(2):
                        if g == 0 and b == 0:
                            # ic=0 has no b=0 block; skip its psum region
                            nc.scalar.activation(
                                pTs[:, 1:4, 0, :],
                                psE[:, 0, P:512],
                                mybir.ActivationFunctionType.Exp,
                                bias=biasj[:, s, 0:1],
                                scale=1.0,
                            )
                        else:
                            nc.scalar.activation(
                                pTs[:, 4 * g : 4 * g + 4, b, :],
                                psE[:, b, :],
                                mybir.ActivationFunctionType.Exp,
                                bias=biasj[:, s, b : b + 1],
                                scale=1.0,
                            )
                # causal mask on all diagonal blocks at once: keep f - p >= 0
                nc.gpsimd.affine_select(
                    pTs[:, :, 1, :],
                    pTs[:, :, 1, :],
                    pattern=[[0, 16], [1, P]],
                    compare_op=mybir.AluOpType.is_ge,
                    fill=0.0,
                    base=0,
                    channel_multiplier=-1,
                )

            def emit_pv_strip(s):
                hp = (s % 2) * 64
                hm = s // 2
                pTs = pT_of.pop(s)
                for g in range(4):
                    pot = psC.tile([65, 512], f32, tag="pot")
                    for u in range(4):
                        ic = 4 * g + u
                        bs = [b for b in range(2) if ic - 1 + b >= 0]
                        for bi, b in enumerate(bs):
                            nc.tensor.matmul(
                                pot[:, u * P : (u + 1) * P],
                                vaug[:, ic - 1 + b, s, 0:65],
                                pTs[:, ic, b, :],
                                start=(bi == 0),
                                stop=(bi == len(bs) - 1),
                            )
                    potsb = work.tile([65, 512], f32, tag="potsb")
                    nc.vector.tensor_copy(potsb[:], pot[:])
                    rs128 = work.tile([P, 4], f32, tag="rs128")
                    nc.sync.dma_start(rs128[:], potsb[64:65, :])
                    nc.vector.reciprocal(rs128[:], rs128[:])
                    srecip = persist.tile([1, 512], f32, tag="srecip")
                    nc.sync.dma_start(srecip[:], rs128[:])
                    bcast = persist.tile([64, 512], f32, tag="bcast")
                    nc.gpsimd.partition_broadcast(bcast[:], srecip[:])
                    nc.vector.tensor_tensor(
                        oT[hp : hp + 64, hm, bass.ts(g, 512)],
                        potsb[0:64, :],
                        bcast[:],
                        mybir.AluOpType.mult,
                    )

            def emit_qk(s):
                W = W_SLOT[s]
                pT = ptp.tile([P, NPTMAX, 512], bf16, tag="pT")
                pT_of[s] = pT
                for jb in range(NJB):
                    c0, c1 = _cr(jb, W)
                    nact = c1 - c0 + 1
                    idx0 = _OFFS[s][jb]
                    # sub-groups of <=2 chunks so QK can run ahead of exp
                    for g0 in range(0, nact, 2):
                        ng = min(2, nact - g0)
                        ssum = psA.tile([P, 2, 512], f32, tag="ssum")
                        for ci in range(ng):
                            c = c0 + g0 + ci
                            nc.tensor.matmul(
                                ssum[:, ci, :],
                                kT2[:, s, bass.ts(jb, P)],
                                qT2[:, s, bass.ts(c, 512)],
                                start=True,
                                stop=True,
                            )
                        nc.scalar.activation(
                            pT[:, idx0 + g0 : idx0 + g0 + ng, :],
                            ssum[:, 0:ng, :],
                            mybir.ActivationFunctionType.Exp,
                            bias=biasj[:, s, jb : jb + 1],
                            scale=1.0,
                        )
                    # causal mask on the diagonal tile (c == c0): keep where
                    # i - j >= 0; i = 512*c0 + f, j = 128*jb + p.  Only the
                    # first 128*(jb%4)+128 columns can violate causality.
                    w = 128 * (jb % 4) + 128
                    nc.gpsimd.affine_select(
                        pT[:, idx0, 0:w],
                        pT[:, idx0, 0:w],
                        pattern=[[1, w]],
                        compare_op=mybir.AluOpType.is_ge,
                        fill=0.0,
                        base=512 * c0 - 128 * jb,
                        channel_multiplier=-1,
                    )

            def emit_pv(s):
                W = W_SLOT[s]
                hp = (s % 2) * 64
                hm = s // 2
                pT = pT_of.pop(s)
                for c in range(NCH):
                    pot = psC.tile([65, 512], f32, tag="pot")
                    jbmin = max(0, -((-(512 * c - W - 127)) // 128))
                    jbs = list(range(jbmin, 4 * c + 4))
                    for ji, jb in enumerate(jbs):
                        nc.tensor.matmul(
                            pot[:],
                            vaug[:, jb, s, 0:65],
                            pT[:, _OFFS[s][jb] + (c - jb // 4), :],
                            start=(ji == 0),
                            stop=(ji == len(jbs) - 1),
                        )
                    # copy out fast to release the PSUM bank, then normalize
                    # off the PV critical path.
                    potsb = work.tile([65, 512], f32, tag="potsb")
                    nc.vector.tensor_copy(potsb[:], pot[:])
                    # spread the 512 rowsums across 128 partitions so the
                    # reciprocal uses all DVE lanes (26ns vs 3.3us)
                    rs128 = work.tile([P, 4], f32, tag="rs128")
                    nc.sync.dma_start(rs128[:], potsb[64:65, :])
                    nc.vector.reciprocal(rs128[:], rs128[:])
                    srecip = persist.tile([1, 512], f32, tag="srecip")
                    nc.sync.dma_start(srecip[:], rs128[:])
                    bcast = persist.tile([64, 512], f32, tag="bcast")
                    nc.gpsimd.partition_broadcast(bcast[:], srecip[:])
                    nc.vector.tensor_tensor(
                        oT[hp : hp + 64, hm, bass.ts(c, 512)],
                        potsb[0:64, :],
                        bcast[:],
                        mybir.AluOpType.mult,
                    )

            def do_qk(s):
                (emit_qk_strip if s < STRIP_SLOTS else emit_qk)(s)

            def do_pv(s):
                (emit_pv_strip if s < STRIP_SLOTS else emit_pv)(s)

            for s in range(NHC):
                do_qk(s)
                if s > 0:
                    do_pv(s - 1)
            do_pv(NHC - 1)

            ptp_cm.__exit__(None, None, None)

            # ---- output projection ----
            with (
                tc.tile_pool(name="wop", bufs=1) as wop,
                tc.tile_pool(name="wst3", bufs=2) as wst3,
                tc.tile_pool(name="ypool", bufs=2) as ypool,
            ):
                wor = wop.tile([P, 4, C], bf16)
                for m in range(4):
                    wo32 = wst3.tile([P, C], f32, tag="wo32")
                    nc.sync.dma_start(wo32[:], wo_r[:, m, :])
                    nc.vector.tensor_copy(wor[:, m, :], wo32[:])

                for tb in range(NJB):
                    # both 512-col halves in one 2-bank psA tile so four
                    # chains stay in flight across the 2 pool bufs
                    psy = psA.tile([P, 2, 512], f32, tag="ssum")
                    for cc in range(2):
                        for m in range(4):
                            nc.tensor.matmul(
                                psy[:, cc, :],
                                oT[:, m, bass.ts(tb, P)],
                                wor[:, m, bass.ts(cc, 512)],
                                start=(m == 0),
                                stop=(m == 3),
                            )
                    ysb = ypool.tile([P, 2, 512], f32, tag="ysb")
                    nc.vector.tensor_copy(ysb[:], psy[:])
                    nc.sync.dma_start(
                        y_r[:, tb, :].rearrange("p (a b) -> p a b", a=2), ysb[:]
                    )

    nc.compile()
    return nc


def kernel(x, Wq, Wk, Wv, Wo):
    global LAST_RESULTS, _NC_CACHE
    x = np.asarray(x, dtype=np.float32)
    Wq = np.asarray(Wq, dtype=np.float32)
    Wk = np.asarray(Wk, dtype=np.float32)
    Wv = np.asarray(Wv, dtype=np.float32)
    Wo = np.asarray(Wo, dtype=np.float32)

    slopes = np.asarray(get_slopes(NH), dtype=np.float64)
    ii = np.arange(T, dtype=np.float64)
    pp = np.arange(P, dtype=np.float64)

    if _NC_CACHE is None:
        _NC_CACHE = build_kernel()
    nc = _NC_CACHE

    import ml_dtypes

    in_maps = []
    for core in range(8):
        b, g = core // 2, core % 2
        heads = [g + 2 * k for k in range(NHC)]
        cols = np.concatenate([np.arange(64 * h, 64 * (h + 1)) for h in heads])
        core_slopes = slopes[heads]

        qaug1 = (-core_slopes[:, None] * ii[None, :]).astype(np.float64)
        # strip slots: per-i offset is -slope*(128 + i mod 128) instead
        qaug1[:STRIP_SLOTS] = -core_slopes[:STRIP_SLOTS, None] * (
            128.0 + (ii[None, :] % 128.0)
        )
        qaug1 = qaug1.astype(ml_dtypes.bfloat16)
        qaugb = np.ascontiguousarray(
            np.broadcast_to(qaug1[:, None, :], (8, NHC, T))
        )
        kaugb = np.zeros((8, NHC, T), ml_dtypes.bfloat16)
        for h in range(NHC):
            kaugb[h, h, :] = ml_dtypes.bfloat16(1.0)
        biasj = np.zeros((P, NHC, NJB), np.float32)
        for h in range(NHC):
            for jb in range(NJB):
                biasj[:, h, jb] = (core_slopes[h] * (128 * jb + pp)).astype(np.float32)
        in_maps.append(
            {
                "xT": np.ascontiguousarray(x[b].T),
                "wq": np.ascontiguousarray(Wq[:, cols]) * np.float32(0.125),
                "wk": np.ascontiguousarray(Wk[:, cols]),
                "wv": np.ascontiguousarray(Wv[:, cols]),
                "wo": np.ascontiguousarray(Wo[cols, :]),
                "qaugb": qaugb,
                "kaugb": kaugb,
                "biasj": biasj,
            }
        )

    res = run_bass_kernel_spmd(nc, in_maps, list(range(8)))
    LAST_RESULTS = res
    out = np.empty((B, T, C), dtype=np.float32)
    for b in range(B):
        out[b] = res.results[2 * b]["y"] + res.results[2 * b + 1]["y"]
    return out


# revision 21
# speedup vs baseline: 1.5813x; 1.0413x over previous
"""Causal attention with ALiBi for nn_CausalAttention (B=4, T=2048, C=1024,
16 heads) on 8 TRN2 NeuronCores.

Sharding: batch (4) x head-parity (2 groups of 8 heads, interleaved so the
two groups have matching slope magnitudes) -> 8 cores.  Core (b, g) handles
heads {g, g+2, ..., g+14} of batch b.

Per core:
  single-pass projections: x chunk loaded once, q/k/v projected per chunk
  (qT/kT in [d, t] layout, v in [t, d]);
  per head-slot s: sT[j, i] = qk/8 + slope*(j - i) via augmented one-hot
  bias matmul plus ACT exp bias of +slope*j; ALiBi decay makes keys beyond
  a per-slot window numerically zero, so score tiles outside the band
  W_SLOT[s] are skipped entirely (QK, exp, and PV); causal masking by
  gpsimd affine_select on the (reduced-width) diagonal tiles; PV with an
  appended ones column gives the softmax denominator; normalization via
  vector.reciprocal + gpsimd partition_broadcast; y_partial = oT.T @ Wo_rows.
Host sums the two head-group partials per batch.
"""

import math

import numpy as np

import concourse.bass as bass
import concourse.mybir as mybir
import concourse.tile as tile
from concourse import bacc
from concourse.bass_utils import run_bass_kernel_spmd

B, T, C = 4, 2048, 1024
NH, HD = 16, 64
NHC = 8  # head-slots per core
BLOCK_SIZE = 2048
NJB = T // 128  # 16 j-blocks
NCH = T // 512  # 4 i-chunks
P = 128

f32 = mybir.dt.float32
bf16 = mybir.dt.bfloat16

LAST_RESULTS = None
_NC_CACHE = None


def get_slopes(n):
    def pow2(n):
        start = 2 ** (-(2 ** (-(math.log2(n) - 3))))
        return [start * start**i for i in range(n)]

    if math.log2(n).is_integer():
        return pow2(n)
    c = 2 ** math.floor(math.log2(n))
    return pow2(c) + get_slopes(2 * c)[0::2][: n - c]


# Per-slot ALiBi windows: slot k holds heads {2k, 2k+1} (parity split), the
# shallower slope s = 0.7071^(2k+2) needs W >= (11 + ln(1/s))/s for the
# dropped softmax mass to stay under ~1e-3.
W_SLOT = [24, 50, 105, 221, 463, 970, 2048, 2048]
# Slots whose window fits in one sub-diagonal 128-block use diagonal-strip
# tiles [128j x 128i] (j in {ic-1, ic} blocks) instead of [128 x 512] tiles:
# 4x less exp area.  Their qaug rows hold -slope*(128 + i mod 128) (the
# per-i offset cancels in softmax), so the exp bias is biasj[:, s, b].
STRIP_SLOTS = 3  # slots 0..2 (W <= 105 < 128)


def _cr(jb, W):
    """Kept i-chunk range [c0, c1] for j-block jb under window W."""
    return jb // 4, min(NCH - 1, (128 * jb + 127 + W) // 512)


# Tile slots are processed in half-ranges of i-chunks (c in {0,1} then
# {2,3}) so each in-flight pT tile holds at most 28 score tiles (vs 40),
# freeing SBUF for the preloaded Wo.
_HOFFS = []  # [slot][half] -> ({jb: (base, clo, chi)}, ntiles)
for _W in W_SLOT:
    _per = []
    for _half in range(2):
        _lo, _hi = (0, 1) if _half == 0 else (2, 3)
        _offs = {}
        _o = 0
        for _jb in range(NJB):
            _c0, _c1 = _cr(_jb, _W)
            _cl, _ch = max(_c0, _lo), min(_c1, _hi)
            if _cl > _ch:
                continue
            _offs[_jb] = (_o, _cl, _ch)
            _o += _ch - _cl + 1
        _per.append((_offs, _o))
    _HOFFS.append(_per)
NPTMAX = max(n for _per in _HOFFS for _, n in _per)  # 28


def build_kernel():
    nc = bacc.Bacc("TRN2", target_bir_lowering=False, debug=False, num_devices=8)

    xT_d = nc.dram_tensor("xT", [C, T], f32, kind="ExternalInput").ap()
    wq_d = nc.dram_tensor("wq", [C, 512], f32, kind="ExternalInput").ap()
    wk_d = nc.dram_tensor("wk", [C, 512], f32, kind="ExternalInput").ap()
    wv_d = nc.dram_tensor("wv", [C, 512], f32, kind="ExternalInput").ap()
    wo_d = nc.dram_tensor("wo", [512, C], f32, kind="ExternalInput").ap()
    qaug_d = nc.dram_tensor("qaugb", [8, NHC, T], bf16, kind="ExternalInput").ap()
    kaug_d = nc.dram_tensor("kaugb", [8, NHC, T], bf16, kind="ExternalInput").ap()
    biasj_d = nc.dram_tensor("biasj", [P, NHC, NJB], f32, kind="ExternalInput").ap()
    y_d = nc.dram_tensor("y", [T, C], f32, kind="ExternalOutput").ap()

    xT_r = xT_d.rearrange("(cb p) t -> p cb t", p=P)  # [128, 8, 2048]
    wq_r = wq_d.rearrange("(cb p) m -> p cb m", p=P)  # [128, 8, 512]
    wk_r = wk_d.rearrange("(cb p) m -> p cb m", p=P)
    wv_r = wv_d.rearrange("(cb p) m -> p cb m", p=P)
    wo_r = wo_d.rearrange("(mb p) n -> p mb n", p=P)  # [128, 4, 1024]
    y_r = y_d.rearrange("(tb p) c -> p tb c", p=P)  # [128, 16, 1024]

    with tile.TileContext(nc) as tc:
        with (
            tc.tile_pool(name="persist", bufs=1) as persist,
            tc.tile_pool(name="work", bufs=2) as work,
            tc.tile_pool(name="psA", bufs=2, space="PSUM") as psA,
            tc.tile_pool(name="psB", bufs=2, space="PSUM") as psB,
            tc.tile_pool(name="psC", bufs=2, space="PSUM") as psC,
        ):
            # ---- persistent tiles ----
            # qT2/kT2: per slot s, rows 0-63 = head data (d), rows 64-71 =
            # augmented bias rows; K=72 matmul contracts both at once.
            qT2 = persist.tile([72, NHC, T], bf16)
            kT2 = persist.tile([72, NHC, T], bf16)
            vaug = persist.tile([P, NJB, NHC, 66], bf16)
            oT = persist.tile([P, 4, T], bf16)
            biasj = persist.tile([P, NHC, NJB], f32)
            wor = persist.tile([P, 4, C], bf16)

            nc.gpsimd.memset(vaug[:, :, :, 64:66], 1.0)
            nc.sync.dma_start(biasj[:], biasj_d[:])
            # aug rows: kT2 row 64+r of slot s is 1.0 iff r == s;
            # qT2 row 64+r of every slot = -slope_r * i
            nc.sync.dma_start(kT2[64:72, :, :], kaug_d[:])
            nc.sync.dma_start(qT2[64:72, :, :], qaug_d[:])

            # ---- single-pass projections ----
            # All weights preloaded (bf16); x streamed chunk by chunk, each
            # chunk feeding v and all four q/k pairs before the next load.
            with (
                tc.tile_pool(name="wts", bufs=1) as wts,
                tc.tile_pool(name="wstage", bufs=4) as wstage,
                tc.tile_pool(name="xp1", bufs=3) as xp1,
                tc.tile_pool(name="xr1", bufs=9) as xr1,
            ):
                wqr = wts.tile([P, 8, 512], bf16, tag="wqr")
                wkr = wts.tile([P, 8, 512], bf16, tag="wkr")
                wvr = wts.tile([P, 8, 512], bf16, tag="wvr")
                # wv first (the first v matmuls only need wvr + x chunk 0);
                # wq/wk/wo spread over the gpsimd queue to parallelize issue
                for c in range(8):
                    w32 = wstage.tile([P, 512], f32, tag="w32")
                    nc.sync.dma_start(w32[:], wv_r[:, c, :])
                    nc.vector.tensor_copy(wvr[:, c, :], w32[:])
                for wsrc, wdst, eng in ((wq_r, wqr, nc.gpsimd), (wk_r, wkr, nc.sync)):
                    for c in range(8):
                        w32 = wstage.tile([P, 512], f32, tag="w32")
                        eng.dma_start(w32[:], wsrc[:, c, :])
                        nc.vector.tensor_copy(wdst[:, c, :], w32[:])
                for m in range(4):
                    for cc in range(2):
                        w32 = wstage.tile([P, 512], f32, tag="w32")
                        nc.gpsimd.dma_start(w32[:], wo_r[:, m, bass.ts(cc, 512)])
                        nc.vector.tensor_copy(wor[:, m, bass.ts(cc, 512)], w32[:])

                for tck in range(NCH):
                    xts = []
                    for c in range(8):
                        x32 = xp1.tile([P, 512], f32, tag="x32")
                        nc.sync.dma_start(x32[:], xT_r[:, c, bass.ts(tck, 512)])
                        xtr = xr1.tile([P, 512], bf16, tag="xtr")
                        nc.vector.tensor_copy(xtr[:], x32[:])
                        xts.append(xtr)
                    # interleave one v chain (psB) with one q/k chain (psA)
                    # per step so the PE never drains a pool dry (keeps the
                    # HAM clock warm: isolated 8-MM chains ran at 1.2 GHz)
                    tsl = bass.ts(tck, 512)
                    for i in range(4):
                        psv = psB.tile([P, 512], f32, tag="pb")
                        for c in range(8):
                            nc.tensor.matmul(
                                psv[:],
                                xts[c][:, bass.ts(i, P)],
                                wvr[:, c, :],
                                start=(c == 0),
                                stop=(c == 7),
                            )
                        nc.vector.tensor_copy(
                            vaug[:, 4 * tck + i, :, 0:64],
                            psv[:].rearrange("p (h d) -> p h d", h=NHC),
                        )
                        psqk = psA.tile([P, 2, 512], f32, tag="ssum")
                        for c in range(8):
                            nc.tensor.matmul(
                                psqk[:, 0, :],
                                wqr[:, c, bass.ts(i, P)],
                                xts[c][:],
                                start=(c == 0),
                                stop=(c == 7),
                            )
                            nc.tensor.matmul(
                                psqk[:, 1, :],
                                wkr[:, c, bass.ts(i, P)],
                                xts[c][:],
                                start=(c == 0),
                                stop=(c == 7),
                            )
                        qkstag = work.tile([P, 2, 512], bf16, tag="qkstag")
                        nc.vector.tensor_copy(qkstag[:], psqk[:])
                        nc.scalar.dma_start(qT2[0:64, 2 * i, tsl], qkstag[0:64, 0, :])
                        nc.scalar.dma_start(qT2[0:64, 2 * i + 1, tsl], qkstag[64:128, 0, :])
                        nc.scalar.dma_start(kT2[0:64, 2 * i, tsl], qkstag[0:64, 1, :])
                        nc.scalar.dma_start(kT2[0:64, 2 * i + 1, tsl], qkstag[64:128, 1, :])

            # ---- attention (banded) ----
            ptp_cm = tc.tile_pool(name="ptp", bufs=2)
            ptp = ptp_cm.__enter__()
            pT_of = {}

            def emit_qk_strip(sh):
                s = sh[0]
                # 16 i-chunks of 128; chunk ic covers j-blocks {ic-1, ic}.
                pTs = work.tile([P, 16, 2, P], bf16, tag="pTs")
                pT_of[sh] = pTs
                for g in range(4):
                    psE = psA.tile([P, 2, 512], f32, tag="ssum")
                    for u in range(4):
                        ic = 4 * g + u
                        for b in range(2):
                            jb = ic - 1 + b
                            if jb < 0:
                                continue
                            nc.tensor.matmul(
                                psE[:, b, u * P : (u + 1) * P],
                                kT2[:, s, bass.ts(jb, P)],
                                qT2[:, s, ic * P : (ic + 1) * P],
                                start=True,
                                stop=True,
                            )
                    for b in range(2):
                        if g == 0 and b == 0:
                            # ic=0 has no b=0 block; skip its psum region
                            nc.scalar.activation(
                                pTs[:, 1:4, 0, :],
                                psE[:, 0, P:512],
                                mybir.ActivationFunctionType.Exp,
                                bias=biasj[:, s, 0:1],
                                scale=1.0,
                            )
                        else:
                            nc.scalar.activation(
                                pTs[:, 4 * g : 4 * g + 4, b, :],
                                psE[:, b, :],
                                mybir.ActivationFunctionType.Exp,
                                bias=biasj[:, s, b : b + 1],
                                scale=1.0,
                            )
                # causal mask on all diagonal blocks at once: keep f - p >= 0
                nc.gpsimd.affine_select(
                    pTs[:, :, 1, :],
                    pTs[:, :, 1, :],
                    pattern=[[0, 16], [1, P]],
                    compare_op=mybir.AluOpType.is_ge,
                    fill=0.0,
                    base=0,
                    channel_multiplier=-1,
                )

            def emit_pv_strip(sh):
                s = sh[0]
                hp = (s % 2) * 64
                hm = s // 2
                pTs = pT_of.pop(sh)
                for g in range(4):
                    pot = psC.tile([65, 512], f32, tag="pot")
                    for u in range(4):
                        ic = 4 * g + u
                        bs = [b for b in range(2) if ic - 1 + b >= 0]
                        for bi, b in enumerate(bs):
                            nc.tensor.matmul(
                                pot[:, u * P : (u + 1) * P],
                                vaug[:, ic - 1 + b, s, 0:65],
                                pTs[:, ic, b, :],
                                start=(bi == 0),
                                stop=(bi == len(bs) - 1),
                            )
                    potsb = work.tile([65, 512], f32, tag="potsb")
                    nc.vector.tensor_copy(potsb[:], pot[:])
                    rs128 = work.tile([P, 4], f32, tag="rs128")
                    nc.sync.dma_start(rs128[:], potsb[64:65, :])
                    nc.vector.reciprocal(rs128[:], rs128[:])
                    srecip = persist.tile([1, 512], f32, tag="srecip")
                    nc.sync.dma_start(srecip[:], rs128[:])
                    bcast = persist.tile([64, 512], f32, tag="bcast")
                    nc.gpsimd.partition_broadcast(bcast[:], srecip[:])
                    nc.vector.tensor_tensor(
                        oT[hp : hp + 64, hm, bass.ts(g, 512)],
                        potsb[0:64, :],
                        bcast[:],
                        mybir.AluOpType.mult,
                    )

            def emit_qk(sh):
                s, half = sh
                offs, _ = _HOFFS[s][half]
                pT = ptp.tile([P, NPTMAX, 512], bf16, tag="pT")
                pT_of[sh] = pT
                for jb, (idx0, cl, ch) in offs.items():
                    nact = ch - cl + 1
                    # sub-groups of <=2 chunks so QK can run ahead of exp
                    for g0 in range(0, nact, 2):
                        ng = min(2, nact - g0)
                        ssum = psA.tile([P, 2, 512], f32, tag="ssum")
                        for ci in range(ng):
                            c = cl + g0 + ci
                            nc.tensor.matmul(
                                ssum[:, ci, :],
                                kT2[:, s, bass.ts(jb, P)],
                                qT2[:, s, bass.ts(c, 512)],
                                start=True,
                                stop=True,
                            )
                        nc.scalar.activation(
                            pT[:, idx0 + g0 : idx0 + g0 + ng, :],
                            ssum[:, 0:ng, :],
                            mybir.ActivationFunctionType.Exp,
                            bias=biasj[:, s, jb : jb + 1],
                            scale=1.0,
                        )
                    # causal mask on the diagonal tile (c == jb//4, in this
                    # half iff cl == jb//4): keep where i - j >= 0;
                    # i = 512*c + f, j = 128*jb + p.  Only the first
                    # 128*(jb%4)+128 columns can violate causality.
                    if cl == jb // 4:
                        w = 128 * (jb % 4) + 128
                        nc.gpsimd.affine_select(
                            pT[:, idx0, 0:w],
                            pT[:, idx0, 0:w],
                            pattern=[[1, w]],
                            compare_op=mybir.AluOpType.is_ge,
                            fill=0.0,
                            base=512 * cl - 128 * jb,
                            channel_multiplier=-1,
                        )

            def emit_pv(sh):
                s, half = sh
                W = W_SLOT[s]
                offs, _ = _HOFFS[s][half]
                hp = (s % 2) * 64
                hm = s // 2
                pT = pT_of.pop(sh)
                for c in ((0, 1) if half == 0 else (2, 3)):
                    pot = psC.tile([65, 512], f32, tag="pot")
                    jbmin = max(0, -((-(512 * c - W - 127)) // 128))
                    jbs = list(range(jbmin, 4 * c + 4))
                    for ji, jb in enumerate(jbs):
                        idx0, cl, ch = offs[jb]
                        nc.tensor.matmul(
                            pot[:],
                            vaug[:, jb, s, 0:65],
                            pT[:, idx0 + (c - cl), :],
                            start=(ji == 0),
                            stop=(ji == len(jbs) - 1),
                        )
                    # copy out fast to release the PSUM bank, then normalize
                    # off the PV critical path.
                    potsb = work.tile([65, 512], f32, tag="potsb")
                    nc.vector.tensor_copy(potsb[:], pot[:])
                    # spread the 512 rowsums across 128 partitions so the
                    # reciprocal uses all DVE lanes (26ns vs 3.3us)
                    rs128 = work.tile([P, 4], f32, tag="rs128")
                    nc.sync.dma_start(rs128[:], potsb[64:65, :])
                    nc.vector.reciprocal(rs128[:], rs128[:])
                    srecip = persist.tile([1, 512], f32, tag="srecip")
                    nc.sync.dma_start(srecip[:], rs128[:])
                    bcast = persist.tile([64, 512], f32, tag="bcast")
                    nc.gpsimd.partition_broadcast(bcast[:], srecip[:])
                    nc.vector.tensor_tensor(
                        oT[hp : hp + 64, hm, bass.ts(c, 512)],
                        potsb[0:64, :],
                        bcast[:],
                        mybir.AluOpType.mult,
                    )
                    if s == NHC - 1:
                        emit_out(c)

            def emit_out(c):
                # output projection for this i-chunk: every slot's oT
                # column block is final once the last slot normalized it
                for tb in range(4 * c, 4 * c + 4):
                    psy = psA.tile([P, 2, 512], f32, tag="ssum")
                    for cc in range(2):
                        for m in range(4):
                            nc.tensor.matmul(
                                psy[:, cc, :],
                                oT[:, m, bass.ts(tb, P)],
                                wor[:, m, bass.ts(cc, 512)],
                                start=(m == 0),
                                stop=(m == 3),
                            )
                    ysb = work.tile([P, 2, 512], f32, tag="ysb")
                    nc.vector.tensor_copy(ysb[:], psy[:])
                    nc.sync.dma_start(
                        y_r[:, tb, :].rearrange("p (a b) -> p a b", a=2), ysb[:]
                    )

            stages = [(s, None) for s in range(STRIP_SLOTS)] + [
                (s, h) for s in range(STRIP_SLOTS, NHC) for h in range(2)
            ]

            def do_qk(sh):
                (emit_qk_strip if sh[1] is None else emit_qk)(sh)

            def do_pv(sh):
                (emit_pv_strip if sh[1] is None else emit_pv)(sh)

            for i, sh in enumerate(stages):
                do_qk(sh)
                if i > 0:
                    do_pv(stages[i - 1])
            do_pv(stages[-1])

            ptp_cm.__exit__(None, None, None)

    nc.compile()
    return nc


def kernel(x, Wq, Wk, Wv, Wo):
    global LAST_RESULTS, _NC_CACHE
    x = np.asarray(x, dtype=np.float32)
    Wq = np.asarray(Wq, dtype=np.float32)
    Wk = np.asarray(Wk, dtype=np.float32)
    Wv = np.asarray(Wv, dtype=np.float32)
    Wo = np.asarray(Wo, dtype=np.float32)

    slopes = np.asarray(get_slopes(NH), dtype=np.float64)
    ii = np.arange(T, dtype=np.float64)
    pp = np.arange(P, dtype=np.float64)

    if _NC_CACHE is None:
        _NC_CACHE = build_kernel()
    nc = _NC_CACHE

    import ml_dtypes

    in_maps = []
    for core in range(8):
        b, g = core // 2, core % 2
        heads = [g + 2 * k for k in range(NHC)]
        cols = np.concatenate([np.arange(64 * h, 64 * (h + 1)) for h in heads])
        core_slopes = slopes[heads]

        qaug1 = (-core_slopes[:, None] * ii[None, :]).astype(np.float64)
        # strip slots: per-i offset is -slope*(128 + i mod 128) instead
        qaug1[:STRIP_SLOTS] = -core_slopes[:STRIP_SLOTS, None] * (
            128.0 + (ii[None, :] % 128.0)
        )
        qaug1 = qaug1.astype(ml_dtypes.bfloat16)
        qaugb = np.ascontiguousarray(
            np.broadcast_to(qaug1[:, None, :], (8, NHC, T))
        )
        kaugb = np.zeros((8, NHC, T), ml_dtypes.bfloat16)
        for h in range(NHC):
            kaugb[h, h, :] = ml_dtypes.bfloat16(1.0)
        biasj = np.zeros((P, NHC, NJB), np.float32)
        for h in range(NHC):
            for jb in range(NJB):
                biasj[:, h, jb] = (core_slopes[h] * (128 * jb + pp)).astype(np.float32)
        in_maps.append(
            {
                "xT": np.ascontiguousarray(x[b].T),
                "wq": np.ascontiguousarray(Wq[:, cols]) * np.float32(0.125),
                "wk": np.ascontiguousarray(Wk[:, cols]),
                "wv": np.ascontiguousarray(Wv[:, cols]),
                "wo": np.ascontiguousarray(Wo[cols, :]),
                "qaugb": qaugb,
                "kaugb": kaugb,
                "biasj": biasj,
            }
        )

    res = run_bass_kernel_spmd(nc, in_maps, list(range(8)))
    LAST_RESULTS = res
    out = np.empty((B, T, C), dtype=np.float32)
    for b in range(B):
        out[b] = res.results[2 * b]["y"] + res.results[2 * b + 1]["y"]
    return out
